# revision 22
# baseline (speedup 1.0000x reference)
"""MoE layer (dense top-2 routing) on 8 Trainium2 NeuronCores.

Sharding: data-parallel over tokens. Each core takes 1024 of the 8192
tokens and computes router logits -> top-2 softmax -> all 8 expert
matmuls -> gated combine for its token slice. No collectives.

Per-core kernel layout:
  xT   [D=1024, T=1024]  token slice, transposed (host-prepped)
  w    [E=8, D=1024, O=1024] expert weights (replicated)
  gwT  [D=1024, E=8]     gate weights transposed
  gb   [128, E=8]        gate bias broadcast over partitions
  eb   [E=8, O=1024]     expert bias
  out  [T=1024, O=1024]

Expert matmuls run as float32r (full PE rate, ~TF32 precision); the
tiny gating matmul runs in plain fp32 so top-2 selection matches the
fp32 reference.
"""

import numpy as np

B, S, D, O, E = 4, 2048, 1024, 1024, 8
NCORES = 8
T = B * S // NCORES
P = 128
KT = D // P          # k tiles over D
TT = T // P          # token tiles per core
OSLICE = 256
OT = O // OSLICE     # output column slices
W_BUFS = 104         # 64 resident W tiles + 40 prefetch


def build_nc(reps=1, egroup=4, tmp_bufs=6, acc_bufs=3):
    import concourse.bacc as bacc
    import concourse.mybir as mybir
    import concourse.tile as tile
    from concourse.masks import make_identity

    f32 = mybir.dt.float32
    f32r = mybir.dt.float32r
    Alu = mybir.AluOpType
    Act = mybir.ActivationFunctionType
    AX = mybir.AxisListType

    nc = bacc.Bacc()
    xT_d = nc.declare_dram_parameter("xT", [D, T], f32, isOutput=False)
    w_d = nc.declare_dram_parameter("w", [E, D, O], f32r, isOutput=False)
    gwT_d = nc.declare_dram_parameter("gwT", [D, E], f32, isOutput=False)
    gb_d = nc.declare_dram_parameter("gb", [P, E], f32, isOutput=False)
    eb_d = nc.declare_dram_parameter("eb", [E, O], f32r, isOutput=False)
    out_d = nc.declare_dram_parameter("out", [T, O], f32, isOutput=True)

    with tile.TileContext(nc) as tc:
        with (
            tc.tile_pool(name="const", bufs=1) as const_pool,
            tc.tile_pool(name="xt", bufs=1) as xt_pool,
            tc.tile_pool(name="wp", bufs=W_BUFS) as w_pool,
            tc.tile_pool(name="sm", bufs=4) as sm_pool,
            tc.tile_pool(name="score", bufs=1) as score_pool,
            tc.tile_pool(name="acc", bufs=acc_bufs) as acc_pool,
            tc.tile_pool(name="tmp", bufs=tmp_bufs) as tmp_pool,
            tc.tile_pool(name="ps", bufs=8, space="PSUM") as ps_pool,
        ):
            ident = const_pool.tile([P, P], f32, tag="ident")
            make_identity(nc, ident[:])
            gb_t = const_pool.tile([P, E], f32, tag="gb")
            nc.sync.dma_start(out=gb_t[:], in_=gb_d[:])
            eb_t = const_pool.tile([E, O], f32r, tag="eb")
            nc.sync.dma_start(out=eb_t[:], in_=eb_d[:])
            gw_t = []
            for k in range(KT):
                g = const_pool.tile([P, E], f32, tag=f"gw{k}")
                nc.sync.dma_start(out=g[:], in_=gwT_d[k * P:(k + 1) * P, :])
                gw_t.append(g)
            # x tiles twice: fp32 for exact gating, f32r for the expert matmuls
            xt = []
            xr = []
            for k in range(KT):
                t = xt_pool.tile([P, T], f32, tag=f"xt{k}", name=f"xt{k}")
                nc.sync.dma_start(out=t[:], in_=xT_d[k * P:(k + 1) * P, :])
                xt.append(t)
                tr = xt_pool.tile([P, T], f32r, tag=f"xr{k}", name=f"xr{k}")
                nc.sync.dma_start(
                    out=tr[:], in_=xT_d[k * P:(k + 1) * P, :].bitcast(f32r))
                xr.append(tr)

            def one_rep():
                # ---- gating: logits -> top-2 mask -> softmax -> scores ----
                score, scoreT = [], []
                for tt in range(TT):
                    tsl = slice(tt * P, (tt + 1) * P)
                    pg = ps_pool.tile([P, E], f32, tag="ps", name="pg")
                    for k in range(KT):
                        nc.tensor.matmul(pg[:], lhsT=xt[k][:, tsl],
                                         rhs=gw_t[k][:],
                                         start=(k == 0), stop=(k == KT - 1))
                    lg = sm_pool.tile([P, E], f32, tag="lg", name="lg")
                    nc.vector.tensor_tensor(lg[:], pg[:], gb_t[:], op=Alu.add)
                    m1 = sm_pool.tile([P, 1], f32, tag="m1", name="m1")
                    nc.vector.tensor_reduce(m1[:], lg[:], axis=AX.X, op=Alu.max)
                    # knock out the argmax, then find the runner-up
                    msk = sm_pool.tile([P, E], f32, tag="msk", name="msk")
                    nc.vector.tensor_scalar(msk[:], lg[:], m1[:], -1e30,
                                            op0=Alu.is_ge, op1=Alu.mult)
                    l2 = sm_pool.tile([P, E], f32, tag="l2", name="l2")
                    nc.vector.tensor_tensor(l2[:], lg[:], msk[:], op=Alu.add)
                    m2 = sm_pool.tile([P, 1], f32, tag="m2", name="m2")
                    nc.vector.tensor_reduce(m2[:], l2[:], axis=AX.X, op=Alu.max)
                    sh = sm_pool.tile([P, E], f32, tag="sh", name="sh")
                    nc.vector.tensor_scalar(sh[:], lg[:], m1[:], None,
                                            op0=Alu.subtract)
                    ex = sm_pool.tile([P, E], f32, tag="ex", name="ex")
                    nc.scalar.activation(ex[:], sh[:], Act.Exp)
                    kp = sm_pool.tile([P, E], f32, tag="kp", name="kp")
                    nc.vector.tensor_scalar(kp[:], lg[:], m2[:], None,
                                            op0=Alu.is_ge)
                    ekp = sm_pool.tile([P, E], f32, tag="ekp", name="ekp")
                    nc.vector.tensor_tensor(ekp[:], ex[:], kp[:], op=Alu.mult)
                    den = sm_pool.tile([P, 1], f32, tag="den", name="den")
                    nc.vector.tensor_reduce(den[:], ekp[:], axis=AX.X,
                                            op=Alu.add)
                    rcp = sm_pool.tile([P, 1], f32, tag="rcp", name="rcp")
                    nc.vector.reciprocal(rcp[:], den[:])
                    sc = score_pool.tile([P, E], f32, tag=f"sc{tt}",
                                         name=f"sc{tt}")
                    nc.vector.tensor_scalar(sc[:], ekp[:], rcp[:], None,
                                            op0=Alu.mult)
                    score.append(sc)
                    pt = ps_pool.tile([E, P], f32, tag="ps", name="pt")
                    nc.tensor.transpose(pt[:], sc[:], ident[:])
                    st = score_pool.tile([E, P], f32r, tag=f"st{tt}",
                                         name=f"st{tt}")
                    nc.vector.tensor_copy(out=st[:], in_=pt[:])
                    scoreT.append(st)

                # ---- experts: fp32r matmuls in PSUM + gated combine ----
                for ot in range(OT):
                    osl = slice(ot * OSLICE, (ot + 1) * OSLICE)
                    wt = {}
                    for k in range(KT):
                        for e in range(E):
                            t = w_pool.tile([P, OSLICE], f32r, tag="w",
                                            name=f"w{k}_{e}")
                            nc.sync.dma_start(
                                out=t[:], in_=w_d[e, k * P:(k + 1) * P, osl])
                            wt[(k, e)] = t
                    for tt in range(TT):
                        tsl = slice(tt * P, (tt + 1) * P)
                        pb = ps_pool.tile([P, OSLICE], f32, tag="ps", name="pb")
                        nc.tensor.matmul(pb[:], lhsT=scoreT[tt][:],
                                         rhs=eb_t[:, osl],
                                         start=True, stop=True)
                        acc = acc_pool.tile([P, OSLICE], f32, tag="acc",
                                            name="acc")
                        # experts in groups: a group's PSUM banks evict while
                        # the next group's matmuls run
                        for g0 in range(0, E, egroup):
                            ges = range(g0, min(g0 + egroup, E))
                            ps = {e: ps_pool.tile([P, OSLICE], f32, tag="ps",
                                                  name=f"pse{e}") for e in ges}
                            for k in range(KT):
                                for e in ges:
                                    nc.tensor.matmul(
                                        ps[e][:],
                                        lhsT=xr[k][:, tsl],
                                        rhs=wt[(k, e)][:],
                                        start=(k == 0), stop=(k == KT - 1))
                            for e in ges:
                                if e == 0:
                                    # first eviction writes acc directly,
                                    # then the bias psum is added from PSUM
                                    nc.scalar.mul(acc[:], ps[e][:],
                                                  mul=score[tt][:, e:e + 1])
                                    nc.vector.tensor_tensor(
                                        acc[:], acc[:], pb[:], op=Alu.add)
                                    continue
                                tmp = tmp_pool.tile([P, OSLICE], f32,
                                                    tag="tmp", name="tmp")
                                nc.scalar.mul(tmp[:], ps[e][:],
                                              mul=score[tt][:, e:e + 1])
                                nc.vector.tensor_tensor(acc[:], acc[:], tmp[:],
                                                        op=Alu.add)
                        nc.sync.dma_start(out=out_d[tsl, osl], in_=acc[:])

            for _rep in range(reps):
                one_rep()

    nc.compile()
    return nc


_cache = {}


def _get_nc():
    if "nc" not in _cache:
        _cache["nc"] = build_nc()
    return _cache["nc"]


def make_in_maps(x, gate_w, gate_b, expert_w, expert_b):
    xflat = np.asarray(x, np.float32).reshape(B * S, D)
    w = np.ascontiguousarray(np.asarray(expert_w, np.float32))
    gwT = np.ascontiguousarray(np.asarray(gate_w, np.float32).T)
    gb = np.ascontiguousarray(
        np.broadcast_to(np.asarray(gate_b, np.float32), (P, E)))
    eb = np.ascontiguousarray(np.asarray(expert_b, np.float32))
    in_maps = []
    for c in range(NCORES):
        xT = np.ascontiguousarray(xflat[c * T:(c + 1) * T].T)
        in_maps.append({"xT": xT, "w": w, "gwT": gwT, "gb": gb, "eb": eb})
    return in_maps


def kernel(x, gate_w, gate_b, expert_w, expert_b):
    from concourse.bass_utils import run_bass_kernel_spmd

    nc = _get_nc()
    in_maps = make_in_maps(x, gate_w, gate_b, expert_w, expert_b)
    res = run_bass_kernel_spmd(nc, in_maps, list(range(NCORES)))
    outs = [res.results[c]["out"] for c in range(NCORES)]
    return np.concatenate(outs, axis=0).reshape(B, S, O)
